# revision 44
# baseline (speedup 1.0000x reference)
"""Trainium2 Bass kernel for nn_CorrProductBlock (equivariant product basis block).

Node-parallel across 8 NeuronCores. Self-contained: hardcodes shapes/sharding.

v2 design notes (vs the original staged baseline):
- Inputs are host-pretransposed to [channel, irrep-group, node] bf16 layout, so
  the device program needs no PE transposes and reads half the HBM bytes.
- Output is produced in the same transposed bf16 layout and host-inverted.
- All big matmuls run as [K<=128] x [128 x TILE_N] bf16 streams; per-element
  weights are gathered with one-hot matmuls (5 streams per tile).
- Elementwise work is balanced across DVE (bf16 2x ops), Act (PSUM evacs) and
  Pool (PSUM-sourced products) to keep every engine under the PE stream time.
- PSUM: h pair-tiles (2 banks, rotating), u (4 banks), gathers (2 banks).
"""

import numpy as np
import ml_dtypes

import concourse.bass as bass
import concourse.bacc as bacc
import concourse.mybir as mybir
import concourse.tile as tile
from concourse.bass_utils import run_bass_kernel_spmd

MUL = 128
NUM_ELEM = 64
N_CORES = 8
TILE_N = 512

F32 = mybir.dt.float32
BF16 = mybir.dt.bfloat16

MULT = mybir.AluOpType.mult
ADD = mybir.AluOpType.add


def _build(ntiles: int, repeat: int = 1):
    """Build the per-core Bass program for `ntiles` tiles of TILE_N nodes.

    repeat>1 wraps the pipeline in a device-side loop (timing amplification)."""
    per_core = ntiles * TILE_N
    nc = bacc.Bacc(num_devices=N_CORES)

    xt = nc.dram_tensor("xt", [128, 4, per_core], BF16, kind="ExternalInput")
    ohb = nc.dram_tensor("ohb", [NUM_ELEM, per_core], BF16, kind="ExternalInput")
    # host-pregathered per-node tables:
    # [0] = w1_0[e], [1] = w2_11[e]*s3, [2] = w1_1[e]
    gq = nc.dram_tensor("gq", [128, 3, per_core], BF16, kind="ExternalInput")
    wnames = ["wpre0", "wpre1", "wsc0", "wsc1", "wco0", "wco1"]
    tnames = ["t200", "t201"]
    wd = {n: nc.dram_tensor(n, [MUL, MUL], BF16, kind="ExternalInput") for n in wnames}
    td = {n: nc.dram_tensor(n, [NUM_ELEM, MUL], BF16, kind="ExternalInput")
          for n in tnames}
    yt = nc.dram_tensor("yt", [128, 4, per_core], BF16, kind="ExternalOutput")

    with tile.TileContext(nc) as tc:
        with (
            tc.tile_pool(name="singles", bufs=1) as singles,
            tc.tile_pool(name="xin", bufs=6) as xin_pool,
            tc.tile_pool(name="ohp", bufs=4) as oh_pool,
            tc.tile_pool(name="chp", bufs=5) as ch_pool,
            tc.tile_pool(name="ewp", bufs=3) as ew_pool,
            tc.tile_pool(name="yp", bufs=3) as y_pool,
            tc.tile_pool(name="ph", bufs=1, space="PSUM") as ph_pool,
            tc.tile_pool(name="pg", bufs=2, space="PSUM") as pg_pool,
            tc.tile_pool(name="pu", bufs=2, space="PSUM") as pu_pool,
        ):
            def load_w(dram, p, tag):
                t = singles.tile([p, MUL], BF16, tag=tag)
                nc.sync.dma_start(out=t, in_=dram[:, :])
                return t

            W = {n: load_w(wd[n], 128, n) for n in wnames}
            T = {n: load_w(td[n], 64, n) for n in tnames}

            env = dict(
                nc=nc, tc=tc, ntiles=ntiles, xt=xt, ohb=ohb, gq=gq, yt=yt,
                xin_pool=xin_pool, oh_pool=oh_pool, ch_pool=ch_pool,
                ew_pool=ew_pool, y_pool=y_pool,
                ph_pool=ph_pool, pg_pool=pg_pool, pu_pool=pu_pool,
                W=W, T=T,
            )

            import contextlib
            rep_ctx = (
                tc.For_i(0, repeat, hint_engines=tuple(mybir.ALL_ENGINES))
                if repeat > 1 else contextlib.nullcontext()
            )
            with rep_ctx:
                _tile_body(env)

    nc.compile()
    return nc


def _tile_body(env):
    nc = env["nc"]
    ntiles = env["ntiles"]
    xt, ohb, gq, yt = env["xt"], env["ohb"], env["gq"], env["yt"]
    xin_pool, oh_pool = env["xin_pool"], env["oh_pool"]
    ch_pool, ew_pool, y_pool = env["ch_pool"], env["ew_pool"], env["y_pool"]
    ph_pool, pg_pool, pu_pool = env["ph_pool"], env["pg_pool"], env["pu_pool"]
    W, T = env["W"], env["T"]

    st = [dict() for _ in range(ntiles)]  # per-tile live tiles

    def bcast3(t):
        # [128, 512] viewed as [128, 3, 512] with middle dim broadcast
        return bass.AP(tensor=t.tensor, offset=t.offset,
                       ap=[t.ap[0], [0, 3], t.ap[1]])

    def s_load(i):
        if not (0 <= i < ntiles):
            return
        ns = slice(i * TILE_N, (i + 1) * TILE_N)
        x = xin_pool.tile([128, 4, TILE_N], BF16, tag="x")
        nc.sync.dma_start(out=x, in_=xt[:, :, ns])
        oh = oh_pool.tile([NUM_ELEM, TILE_N], BF16, tag="oh")
        nc.sync.dma_start(out=oh, in_=ohb[:, ns])
        g = oh_pool.tile([128, 3, TILE_N], BF16, tag="gq")
        nc.sync.dma_start(out=g, in_=gq[:, :, ns])
        st[i]["x"], st[i]["oh"], st[i]["gq"] = x, oh, g

    def s_h(i):
        if not (0 <= i < ntiles):
            return
        x = st[i]["x"]
        h = ph_pool.tile([128, 4, TILE_N], F32, tag="h")
        nc.tensor.matmul(h[:, 0, :], W["wpre0"], x[:, 0, :], start=True, stop=True)
        for j in range(3):
            nc.tensor.matmul(h[:, 1 + j, :], W["wpre1"], x[:, 1 + j, :],
                             start=True, stop=True)
        st[i]["h"] = h

    def s_ch(i):
        if not (0 <= i < ntiles):
            return
        ch = ch_pool.tile([128, 4, TILE_N], BF16, tag="ch")
        nc.scalar.copy(out=ch, in_=st[i]["h"])
        st[i]["ch"] = ch

    def s_gA(i):
        # g200, g201 one-hot gathers into a pu-pool pair tile (time-shared)
        if not (0 <= i < ntiles):
            return
        oh = st[i]["oh"]
        gp = pu_pool.tile([128, 2, TILE_N], F32, tag="u")
        nc.tensor.matmul(gp[:, 0, :], T["t200"], oh, start=True, stop=True)
        nc.tensor.matmul(gp[:, 1, :], T["t201"], oh, start=True, stop=True)
        st[i]["g200"], st[i]["g201"] = gp[:, 0, :], gp[:, 1, :]

    def s_t1(i):
        if not (0 <= i < ntiles):
            return
        ch = st[i]["ch"]
        t1 = ew_pool.tile([128, TILE_N], BF16, tag="t1")
        nc.vector.tensor_tensor(out=t1, in0=st[i]["g200"], in1=ch[:, 0, :], op=MULT)
        st[i]["t1"] = t1

    def s_p1(i):
        if not (0 <= i < ntiles):
            return
        ch = st[i]["ch"]
        p1 = ew_pool.tile([128, TILE_N], BF16, tag="p1")
        nc.vector.tensor_tensor(out=p1, in0=st[i]["g201"], in1=ch[:, 0, :], op=MULT)
        st[i]["p1"] = p1

    def s_u01(i):
        if not (0 <= i < ntiles):
            return
        x = st[i]["x"]
        a0a, z, a1 = st[i]["a0a"], st[i]["z"], st[i]["a1"]
        u = pu_pool.tile([128, 2, TILE_N], F32, tag="u")
        nc.tensor.matmul(u[:, 0, :], W["wsc0"], x[:, 0, :], start=True, stop=False)
        nc.tensor.matmul(u[:, 0, :], W["wco0"], a0a, start=False, stop=False)
        nc.tensor.matmul(u[:, 0, :], W["wco0"], z, start=False, stop=True)
        nc.tensor.matmul(u[:, 1, :], W["wsc1"], x[:, 1, :], start=True, stop=False)
        nc.tensor.matmul(u[:, 1, :], W["wco1"], a1[:, 0, :], start=False, stop=True)
        st[i]["u01"] = u

    def s_u23(i):
        if not (0 <= i < ntiles):
            return
        x = st[i]["x"]
        a1 = st[i]["a1"]
        u = pu_pool.tile([128, 2, TILE_N], F32, tag="u")
        for j in (1, 2):
            nc.tensor.matmul(u[:, j - 1, :], W["wsc1"], x[:, 1 + j, :],
                             start=True, stop=False)
            nc.tensor.matmul(u[:, j - 1, :], W["wco1"], a1[:, j, :],
                             start=False, stop=True)
        st[i]["u23"] = u

    def s_y01(i):
        if not (0 <= i < ntiles):
            return
        y = y_pool.tile([128, 4, TILE_N], BF16, tag="y")
        nc.scalar.copy(out=y[:, 0:2, :], in_=st[i]["u01"])
        st[i]["y"] = y

    def s_y23(i):
        if not (0 <= i < ntiles):
            return
        nc.scalar.copy(out=st[i]["y"][:, 2:4, :], in_=st[i]["u23"])

    def s_out(i):
        if not (0 <= i < ntiles):
            return
        ns = slice(i * TILE_N, (i + 1) * TILE_N)
        nc.sync.dma_start(out=yt[:, :, ns], in_=st[i]["y"])
        st[i].clear()

    def s_sq(i):
        if not (0 <= i < ntiles):
            return
        ch = st[i]["ch"]
        sq = ew_pool.tile([128, 3, TILE_N], BF16, tag="sq")
        nc.vector.tensor_tensor(out=sq, in0=ch[:, 1:4, :], in1=ch[:, 1:4, :],
                                op=MULT)
        st[i]["sq"] = sq

    def s_ss(i):
        if not (0 <= i < ntiles):
            return
        sq = st[i]["sq"]
        ssX = ew_pool.tile([128, TILE_N], BF16, tag="ssX")
        nc.gpsimd.tensor_tensor(out=ssX, in0=sq[:, 0, :], in1=sq[:, 1, :], op=ADD)
        ss = ew_pool.tile([128, TILE_N], BF16, tag="ss")
        nc.gpsimd.tensor_tensor(out=ss, in0=ssX, in1=sq[:, 2, :], op=ADD)
        st[i]["ss"] = ss

    def s_t2(i):
        if not (0 <= i < ntiles):
            return
        t2 = ew_pool.tile([128, TILE_N], BF16, tag="t2")
        nc.vector.tensor_tensor(out=t2, in0=st[i]["gq"][:, 0, :], in1=st[i]["t1"],
                                op=ADD)
        st[i]["t2"] = t2

    def s_z(i):
        if not (0 <= i < ntiles):
            return
        z = ew_pool.tile([128, TILE_N], BF16, tag="z")
        nc.vector.tensor_tensor(out=z, in0=st[i]["gq"][:, 1, :], in1=st[i]["ss"],
                                op=MULT)
        st[i]["z"] = z

    def s_a0a(i):
        if not (0 <= i < ntiles):
            return
        ch = st[i]["ch"]
        a0a = ew_pool.tile([128, TILE_N], BF16, tag="a0a")
        nc.gpsimd.tensor_tensor(out=a0a, in0=ch[:, 0, :], in1=st[i]["t2"], op=MULT)
        st[i]["a0a"] = a0a

    def s_p2(i):
        if not (0 <= i < ntiles):
            return
        p2 = ew_pool.tile([128, TILE_N], BF16, tag="p2")
        nc.vector.tensor_tensor(out=p2, in0=st[i]["gq"][:, 2, :], in1=st[i]["p1"],
                                op=ADD)
        st[i]["p2"] = p2

    def s_a1(i):
        if not (0 <= i < ntiles):
            return
        a1 = ew_pool.tile([128, 3, TILE_N], BF16, tag="a1")
        nc.vector.tensor_tensor(out=a1, in0=bcast3(st[i]["p2"]),
                                in1=st[i]["ch"][:, 1:4, :], op=MULT)
        st[i]["a1"] = a1

    # Software-pipelined emission: per-engine streams are FIFO in program
    # order; tile i's u/y/store trail by two iterations so the elementwise
    # chain of tile i never stalls the PE's u-streams.
    s_load(0)
    s_load(1)
    for i in range(ntiles + 2):
        s_load(i + 2)
        s_h(i)
        s_ch(i)
        s_u01(i - 2)
        s_y01(i - 2)
        s_u23(i - 2)
        s_y23(i - 2)
        s_gA(i)
        s_t1(i)
        s_p1(i)
        s_out(i - 2)
        s_t2(i)
        s_p2(i)
        s_sq(i)
        s_ss(i)
        s_z(i)
        s_a0a(i)
        s_a1(i)


# ---------------------------------------------------------------------------
# Host-side data prep


def _prep_weights(inp):
    s = 1.0 / np.sqrt(MUL)
    s3 = 1.0 / np.sqrt(3.0)
    f = lambda a: np.asarray(a, dtype=np.float32)
    bf = lambda a: np.ascontiguousarray(a.astype(ml_dtypes.bfloat16))
    w = {}
    w["wpre0"] = bf(f(inp["Wpre0"]) * s)
    w["wpre1"] = bf(f(inp["Wpre1"]) * s)
    w["wco0"] = bf((f(inp["Wprod0"]) @ f(inp["Wout0"])) * (s * s))
    w["wco1"] = bf((f(inp["Wprod1"]) @ f(inp["Wout1"])) * (s * s))
    w["wsc0"] = bf(f(inp["Wsc0"]) * s)
    w["wsc1"] = bf(f(inp["Wsc1"]) * s)
    w["t200"] = bf(f(inp["w2_00"]))
    w["t201"] = bf(f(inp["w2_01"]))
    # host-gathered tables (built per-node in run_sharded): [64, 3, 128]
    gtabs = np.stack(
        [f(inp["w1_0"]), f(inp["w2_11"]) * s3, f(inp["w1_1"])], axis=1
    )
    return w, gtabs.astype(ml_dtypes.bfloat16)


def _transpose_in(node_feats_bf, n_pad):
    """[n, 512] bf16 -> [128, 4, n_pad] bf16 (channel, group, node)."""
    n = node_feats_bf.shape[0]
    out = np.zeros((128, 4, n_pad), dtype=ml_dtypes.bfloat16)
    out[:, 0, :n] = node_feats_bf[:, :MUL].T
    vec = node_feats_bf[:, MUL:].reshape(n, MUL, 3)
    out[:, 1:4, :n] = vec.transpose(1, 2, 0)
    return out


def _transpose_out(y, n):
    """[128, 4, n_pad] bf16 -> [n, 512] f32."""
    out = np.empty((n, 512), dtype=np.float32)
    out[:, :MUL] = y[:, 0, :n].T
    out[:, MUL:] = y[:, 1:4, :n].transpose(2, 0, 1).reshape(n, 3 * MUL)
    return out


_cache = {}


def _get_program(ntiles):
    if ntiles not in _cache:
        _cache[ntiles] = _build(ntiles)
    return _cache[ntiles]


def _bench_in_maps(ntiles, seed=0):
    """Random, correctly-shaped inputs for timing programs."""
    per_core = ntiles * TILE_N
    rng = np.random.default_rng(seed)
    bf = ml_dtypes.bfloat16
    base = {
        "xt": rng.standard_normal((128, 4, per_core), dtype=np.float32).astype(bf),
        "ohb": np.zeros((NUM_ELEM, per_core), dtype=bf),
        "gq": rng.standard_normal((128, 3, per_core), dtype=np.float32).astype(bf),
    }
    e = rng.integers(0, NUM_ELEM, size=per_core)
    base["ohb"][e, np.arange(per_core)] = 1.0
    for n in ["wpre0", "wpre1", "wsc0", "wsc1", "wco0", "wco1"]:
        base[n] = rng.standard_normal((MUL, MUL), dtype=np.float32).astype(bf)
    for n in ["t200", "t201"]:
        base[n] = rng.standard_normal((NUM_ELEM, MUL), dtype=np.float32).astype(bf)
    return [dict(base) for _ in range(N_CORES)]


def run_sharded(node_feats, node_elems, weights, gtabs, n_nodes, trace=False):
    """Run on hardware: shard n_nodes across 8 cores, pad to tile multiple."""
    per_core_raw = (n_nodes + N_CORES - 1) // N_CORES
    ntiles = (per_core_raw + TILE_N - 1) // TILE_N
    per_core = ntiles * TILE_N

    nf_bf = node_feats.astype(ml_dtypes.bfloat16)
    in_maps = []
    counts = []
    for c in range(N_CORES):
        lo = c * per_core_raw
        hi = min(n_nodes, lo + per_core_raw)
        cnt = max(0, hi - lo)
        counts.append(cnt)
        xt = _transpose_in(nf_bf[lo:hi], per_core)
        ohb = np.zeros((NUM_ELEM, per_core), dtype=ml_dtypes.bfloat16)
        gq = np.zeros((128, 3, per_core), dtype=ml_dtypes.bfloat16)
        if cnt:
            e = np.asarray(node_elems[lo:hi]).astype(np.int64)
            ohb[e, np.arange(cnt)] = 1.0
            # gtabs [64, 3, 128] -> per-node [cnt, 3, 128] -> [128, 3, cnt]
            gq[:, :, :cnt] = gtabs[e].transpose(2, 1, 0)
        in_maps.append({"xt": xt, "ohb": ohb, "gq": gq, **weights})

    nc = _get_program(ntiles)
    res = run_bass_kernel_spmd(
        nc, in_maps, core_ids=list(range(N_CORES)), trace=trace
    )
    out = np.empty((n_nodes, 512), dtype=np.float32)
    for c in range(N_CORES):
        lo = c * per_core_raw
        if counts[c]:
            out[lo:lo + counts[c]] = _transpose_out(res.results[c]["yt"], counts[c])
    return out, res


def kernel(**inputs):
    inputs = {k: np.asarray(v) for k, v in inputs.items()}
    node_feats = inputs["node_feats"].astype(np.float32, copy=False)
    node_elems = inputs["node_elems"]
    weights, gtabs = _prep_weights(inputs)
    out, _ = run_sharded(node_feats, node_elems, weights, gtabs,
                         node_feats.shape[0])
    return out


# revision 65
# speedup vs baseline: 1.2384x; 1.2384x over previous
"""Trainium2 Bass kernel for nn_CorrProductBlock (equivariant product basis block).

Node-parallel across 8 NeuronCores. Self-contained: hardcodes shapes/sharding.

v2 design notes (vs the original staged baseline):
- Inputs are host-pretransposed to [channel, irrep-group, node] bf16 layout, so
  the device program needs no PE transposes and reads half the HBM bytes.
- Output is produced in the same transposed bf16 layout and host-inverted.
- All big matmuls run as [K<=128] x [128 x TILE_N] bf16 streams; per-element
  weights are gathered with one-hot matmuls (5 streams per tile).
- Elementwise work is balanced across DVE (bf16 2x ops), Act (PSUM evacs) and
  Pool (PSUM-sourced products) to keep every engine under the PE stream time.
- PSUM: h pair-tiles (2 banks, rotating), u (4 banks), gathers (2 banks).
"""

import numpy as np
import ml_dtypes

import concourse.bass as bass
import concourse.bacc as bacc
import concourse.mybir as mybir
import concourse.tile as tile
from concourse.bass_utils import run_bass_kernel_spmd

MUL = 128
NUM_ELEM = 64
N_CORES = 8
TILE_N = 512

F32 = mybir.dt.float32
BF16 = mybir.dt.bfloat16

MULT = mybir.AluOpType.mult
ADD = mybir.AluOpType.add


def _build(ntiles: int, repeat: int = 1):
    """Build the per-core Bass program for `ntiles` tiles of TILE_N nodes.

    repeat>1 wraps the pipeline in a device-side loop (timing amplification)."""
    per_core = ntiles * TILE_N
    nc = bacc.Bacc(num_devices=N_CORES)

    # Per-tile contiguous input block: rows 0-3 = x channels (x0, x1a..c),
    # rows 4-6 = host-gathered tables (w1_0[e], w2_11[e]*s3, w1_1[e]),
    # row 7 = one-hot of node_elems on partitions 0-63 (rest zero).
    xg = nc.dram_tensor("xg", [ntiles, 128, 8, TILE_N], BF16,
                        kind="ExternalInput")
    wnames = ["wpre0", "wpre1", "wsc0", "wsc1", "wco0", "wco1"]
    tnames = ["t200", "t201"]
    wd = {n: nc.dram_tensor(n, [MUL, MUL], BF16, kind="ExternalInput") for n in wnames}
    td = {n: nc.dram_tensor(n, [NUM_ELEM, MUL], BF16, kind="ExternalInput")
          for n in tnames}
    yt = nc.dram_tensor("yt", [ntiles, 128, 4, TILE_N], BF16,
                        kind="ExternalOutput")

    with tile.TileContext(nc) as tc:
        with (
            tc.tile_pool(name="singles", bufs=1) as singles,
            tc.tile_pool(name="xin", bufs=6) as xin_pool,
            tc.tile_pool(name="ohp", bufs=4) as oh_pool,
            tc.tile_pool(name="chp", bufs=5) as ch_pool,
            tc.tile_pool(name="ewp", bufs=3) as ew_pool,
            tc.tile_pool(name="yp", bufs=3) as y_pool,
            tc.tile_pool(name="ph", bufs=1, space="PSUM") as ph_pool,
            tc.tile_pool(name="pg", bufs=2, space="PSUM") as pg_pool,
            tc.tile_pool(name="pu", bufs=2, space="PSUM") as pu_pool,
        ):
            def load_w(dram, p, tag):
                t = singles.tile([p, MUL], BF16, tag=tag)
                nc.sync.dma_start(out=t, in_=dram[:, :])
                return t

            W = {n: load_w(wd[n], 128, n) for n in wnames}
            T = {n: load_w(td[n], 64, n) for n in tnames}

            env = dict(
                nc=nc, tc=tc, ntiles=ntiles, xg=xg, yt=yt,
                xin_pool=xin_pool, oh_pool=oh_pool, ch_pool=ch_pool,
                ew_pool=ew_pool, y_pool=y_pool,
                ph_pool=ph_pool, pg_pool=pg_pool, pu_pool=pu_pool,
                W=W, T=T,
            )

            import contextlib
            rep_ctx = (
                tc.For_i(0, repeat, hint_engines=tuple(mybir.ALL_ENGINES))
                if repeat > 1 else contextlib.nullcontext()
            )
            with rep_ctx:
                _tile_body(env)

    nc.compile()
    return nc


def _tile_body(env):
    nc = env["nc"]
    ntiles = env["ntiles"]
    xg, yt = env["xg"], env["yt"]
    xin_pool, oh_pool = env["xin_pool"], env["oh_pool"]
    ch_pool, ew_pool, y_pool = env["ch_pool"], env["ew_pool"], env["y_pool"]
    ph_pool, pg_pool, pu_pool = env["ph_pool"], env["pg_pool"], env["pu_pool"]
    W, T = env["W"], env["T"]

    st = [dict() for _ in range(ntiles)]  # per-tile live tiles

    def bcast3(t):
        # [128, 512] viewed as [128, 3, 512] with middle dim broadcast
        return bass.AP(tensor=t.tensor, offset=t.offset,
                       ap=[t.ap[0], [0, 3], t.ap[1]])

    def s_load(i):
        if not (0 <= i < ntiles):
            return
        blk = xin_pool.tile([128, 8, TILE_N], BF16, tag="x")
        nc.sync.dma_start(out=blk, in_=xg[i])
        st[i]["x"] = blk[:, 0:4, :]
        st[i]["gq"] = blk[:, 4:7, :]
        st[i]["oh"] = blk[0:NUM_ELEM, 7, :]

    def s_h(i):
        if not (0 <= i < ntiles):
            return
        x = st[i]["x"]
        h = ph_pool.tile([128, 4, TILE_N], F32, tag="h")
        nc.tensor.matmul(h[:, 0, :], W["wpre0"], x[:, 0, :], start=True, stop=True)
        for j in range(3):
            nc.tensor.matmul(h[:, 1 + j, :], W["wpre1"], x[:, 1 + j, :],
                             start=True, stop=True)
        st[i]["h"] = h

    def s_ch(i):
        if not (0 <= i < ntiles):
            return
        ch = ch_pool.tile([128, 4, TILE_N], BF16, tag="ch")
        nc.scalar.copy(out=ch, in_=st[i]["h"])
        st[i]["ch"] = ch

    def s_gA(i):
        # g200, g201 one-hot gathers into a pu-pool pair tile (time-shared)
        if not (0 <= i < ntiles):
            return
        oh = st[i]["oh"]
        gp = pu_pool.tile([128, 2, TILE_N], F32, tag="u")
        nc.tensor.matmul(gp[:, 0, :], T["t200"], oh, start=True, stop=True)
        nc.tensor.matmul(gp[:, 1, :], T["t201"], oh, start=True, stop=True)
        st[i]["g200"], st[i]["g201"] = gp[:, 0, :], gp[:, 1, :]

    def s_t1(i):
        if not (0 <= i < ntiles):
            return
        ch = st[i]["ch"]
        t1 = ew_pool.tile([128, TILE_N], BF16, tag="t1")
        nc.vector.tensor_tensor(out=t1, in0=st[i]["g200"], in1=ch[:, 0, :], op=MULT)
        st[i]["t1"] = t1

    def s_p1(i):
        if not (0 <= i < ntiles):
            return
        ch = st[i]["ch"]
        p1 = ew_pool.tile([128, TILE_N], BF16, tag="p1")
        nc.vector.tensor_tensor(out=p1, in0=st[i]["g201"], in1=ch[:, 0, :], op=MULT)
        st[i]["p1"] = p1

    def s_u01(i):
        if not (0 <= i < ntiles):
            return
        x = st[i]["x"]
        a0a, z, a1 = st[i]["a0a"], st[i]["z"], st[i]["a1"]
        u = pu_pool.tile([128, 2, TILE_N], F32, tag="u")
        nc.tensor.matmul(u[:, 0, :], W["wsc0"], x[:, 0, :], start=True, stop=False)
        nc.tensor.matmul(u[:, 0, :], W["wco0"], a0a, start=False, stop=False)
        nc.tensor.matmul(u[:, 0, :], W["wco0"], z, start=False, stop=True)
        nc.tensor.matmul(u[:, 1, :], W["wsc1"], x[:, 1, :], start=True, stop=False)
        nc.tensor.matmul(u[:, 1, :], W["wco1"], a1[:, 0, :], start=False, stop=True)
        st[i]["u01"] = u

    def s_u23(i):
        if not (0 <= i < ntiles):
            return
        x = st[i]["x"]
        a1 = st[i]["a1"]
        u = pu_pool.tile([128, 2, TILE_N], F32, tag="u")
        for j in (1, 2):
            nc.tensor.matmul(u[:, j - 1, :], W["wsc1"], x[:, 1 + j, :],
                             start=True, stop=False)
            nc.tensor.matmul(u[:, j - 1, :], W["wco1"], a1[:, j, :],
                             start=False, stop=True)
        st[i]["u23"] = u

    def s_y01(i):
        if not (0 <= i < ntiles):
            return
        y = y_pool.tile([128, 4, TILE_N], BF16, tag="y")
        nc.scalar.copy(out=y[:, 0:2, :], in_=st[i]["u01"])
        st[i]["y"] = y

    def s_y23(i):
        if not (0 <= i < ntiles):
            return
        nc.scalar.copy(out=st[i]["y"][:, 2:4, :], in_=st[i]["u23"])

    def s_out(i):
        if not (0 <= i < ntiles):
            return
        nc.sync.dma_start(out=yt[i], in_=st[i]["y"])
        st[i].clear()

    def s_sq(i):
        if not (0 <= i < ntiles):
            return
        ch = st[i]["ch"]
        sq = ew_pool.tile([128, 3, TILE_N], BF16, tag="sq")
        nc.vector.tensor_tensor(out=sq, in0=ch[:, 1:4, :], in1=ch[:, 1:4, :],
                                op=MULT)
        st[i]["sq"] = sq

    def s_ss(i):
        if not (0 <= i < ntiles):
            return
        sq = st[i]["sq"]
        ssX = ew_pool.tile([128, TILE_N], BF16, tag="ssX")
        nc.gpsimd.tensor_tensor(out=ssX, in0=sq[:, 0, :], in1=sq[:, 1, :], op=ADD)
        ss = ew_pool.tile([128, TILE_N], BF16, tag="ss")
        nc.gpsimd.tensor_tensor(out=ss, in0=ssX, in1=sq[:, 2, :], op=ADD)
        st[i]["ss"] = ss

    def s_t2(i):
        if not (0 <= i < ntiles):
            return
        t2 = ew_pool.tile([128, TILE_N], BF16, tag="t2")
        nc.vector.tensor_tensor(out=t2, in0=st[i]["gq"][:, 0, :], in1=st[i]["t1"],
                                op=ADD)
        st[i]["t2"] = t2

    def s_z(i):
        if not (0 <= i < ntiles):
            return
        z = ew_pool.tile([128, TILE_N], BF16, tag="z")
        nc.vector.tensor_tensor(out=z, in0=st[i]["gq"][:, 1, :], in1=st[i]["ss"],
                                op=MULT)
        st[i]["z"] = z

    def s_a0a(i):
        if not (0 <= i < ntiles):
            return
        ch = st[i]["ch"]
        a0a = ew_pool.tile([128, TILE_N], BF16, tag="a0a")
        nc.gpsimd.tensor_tensor(out=a0a, in0=ch[:, 0, :], in1=st[i]["t2"], op=MULT)
        st[i]["a0a"] = a0a

    def s_p2(i):
        if not (0 <= i < ntiles):
            return
        p2 = ew_pool.tile([128, TILE_N], BF16, tag="p2")
        nc.vector.tensor_tensor(out=p2, in0=st[i]["gq"][:, 2, :], in1=st[i]["p1"],
                                op=ADD)
        st[i]["p2"] = p2

    def s_a1(i):
        if not (0 <= i < ntiles):
            return
        a1 = ew_pool.tile([128, 3, TILE_N], BF16, tag="a1")
        nc.vector.tensor_tensor(out=a1, in0=bcast3(st[i]["p2"]),
                                in1=st[i]["ch"][:, 1:4, :], op=MULT)
        st[i]["a1"] = a1

    # Software-pipelined emission: per-engine streams are FIFO in program
    # order; tile i's u/y/store trail by two iterations so the elementwise
    # chain of tile i never stalls the PE's u-streams.
    s_load(0)
    s_load(1)
    for i in range(ntiles + 2):
        s_load(i + 2)
        s_h(i)
        s_ch(i)
        s_u01(i - 2)
        s_y01(i - 2)
        s_u23(i - 2)
        s_y23(i - 2)
        s_gA(i)
        s_t1(i)
        s_p1(i)
        s_out(i - 2)
        s_t2(i)
        s_p2(i)
        s_a1(i)
        s_sq(i)
        s_ss(i)
        s_z(i)
        s_a0a(i)


# ---------------------------------------------------------------------------
# Host-side data prep


def _prep_weights(inp):
    s = 1.0 / np.sqrt(MUL)
    s3 = 1.0 / np.sqrt(3.0)
    f = lambda a: np.asarray(a, dtype=np.float32)
    bf = lambda a: np.ascontiguousarray(a.astype(ml_dtypes.bfloat16))
    w = {}
    w["wpre0"] = bf(f(inp["Wpre0"]) * s)
    w["wpre1"] = bf(f(inp["Wpre1"]) * s)
    w["wco0"] = bf((f(inp["Wprod0"]) @ f(inp["Wout0"])) * (s * s))
    w["wco1"] = bf((f(inp["Wprod1"]) @ f(inp["Wout1"])) * (s * s))
    w["wsc0"] = bf(f(inp["Wsc0"]) * s)
    w["wsc1"] = bf(f(inp["Wsc1"]) * s)
    w["t200"] = bf(f(inp["w2_00"]))
    w["t201"] = bf(f(inp["w2_01"]))
    # host-gathered tables (built per-node in run_sharded): [64, 3, 128]
    gtabs = np.stack(
        [f(inp["w1_0"]), f(inp["w2_11"]) * s3, f(inp["w1_1"])], axis=1
    )
    return w, gtabs.astype(ml_dtypes.bfloat16)


def _transpose_in(node_feats_bf, n_pad):
    """[n, 512] bf16 -> [128, 4, n_pad] bf16 (channel, group, node)."""
    n = node_feats_bf.shape[0]
    out = np.zeros((128, 4, n_pad), dtype=ml_dtypes.bfloat16)
    out[:, 0, :n] = node_feats_bf[:, :MUL].T
    vec = node_feats_bf[:, MUL:].reshape(n, MUL, 3)
    out[:, 1:4, :n] = vec.transpose(1, 2, 0)
    return out


def _transpose_out(y, n):
    """[128, 4, n_pad] bf16 -> [n, 512] f32."""
    out = np.empty((n, 512), dtype=np.float32)
    out[:, :MUL] = y[:, 0, :n].T
    out[:, MUL:] = y[:, 1:4, :n].transpose(2, 0, 1).reshape(n, 3 * MUL)
    return out


_cache = {}


def _get_program(ntiles):
    if ntiles not in _cache:
        _cache[ntiles] = _build(ntiles)
    return _cache[ntiles]


def _bench_in_maps(ntiles, seed=0):
    """Random, correctly-shaped inputs for timing programs."""
    per_core = ntiles * TILE_N
    rng = np.random.default_rng(seed)
    bf = ml_dtypes.bfloat16
    xg = rng.standard_normal((ntiles, 128, 8, TILE_N), dtype=np.float32).astype(bf)
    xg[:, :, 7, :] = 0
    e = rng.integers(0, NUM_ELEM, size=per_core).reshape(ntiles, TILE_N)
    for t in range(ntiles):
        xg[t, e[t], 7, np.arange(TILE_N)] = 1.0
    base = {"xg": xg}
    for n in ["wpre0", "wpre1", "wsc0", "wsc1", "wco0", "wco1"]:
        base[n] = rng.standard_normal((MUL, MUL), dtype=np.float32).astype(bf)
    for n in ["t200", "t201"]:
        base[n] = rng.standard_normal((NUM_ELEM, MUL), dtype=np.float32).astype(bf)
    return [dict(base) for _ in range(N_CORES)]


def run_sharded(node_feats, node_elems, weights, gtabs, n_nodes, trace=False):
    """Run on hardware: shard n_nodes across 8 cores, pad to tile multiple."""
    per_core_raw = (n_nodes + N_CORES - 1) // N_CORES
    ntiles = (per_core_raw + TILE_N - 1) // TILE_N
    per_core = ntiles * TILE_N

    nf_bf = node_feats.astype(ml_dtypes.bfloat16)
    in_maps = []
    counts = []
    for c in range(N_CORES):
        lo = c * per_core_raw
        hi = min(n_nodes, lo + per_core_raw)
        cnt = max(0, hi - lo)
        counts.append(cnt)
        # [128, rows, per_core] planes, then fold to tile-major [nt,128,8,T]
        planes = np.zeros((128, 8, per_core), dtype=ml_dtypes.bfloat16)
        planes[:, 0:4, :] = _transpose_in(nf_bf[lo:hi], per_core)
        if cnt:
            e = np.asarray(node_elems[lo:hi]).astype(np.int64)
            # gtabs [64, 3, 128] -> per-node [cnt, 3, 128] -> [128, 3, cnt]
            planes[:, 4:7, :cnt] = gtabs[e].transpose(2, 1, 0)
            planes[e, 7, np.arange(cnt)] = 1.0
        xg = np.ascontiguousarray(
            planes.reshape(128, 8, ntiles, TILE_N).transpose(2, 0, 1, 3)
        )
        in_maps.append({"xg": xg, **weights})

    nc = _get_program(ntiles)
    res = run_bass_kernel_spmd(
        nc, in_maps, core_ids=list(range(N_CORES)), trace=trace
    )
    out = np.empty((n_nodes, 512), dtype=np.float32)
    for c in range(N_CORES):
        lo = c * per_core_raw
        if counts[c]:
            y = res.results[c]["yt"]  # [nt, 128, 4, T]
            y = y.transpose(1, 2, 0, 3).reshape(128, 4, per_core)
            out[lo:lo + counts[c]] = _transpose_out(y, counts[c])
    return out, res


def kernel(**inputs):
    inputs = {k: np.asarray(v) for k, v in inputs.items()}
    node_feats = inputs["node_feats"].astype(np.float32, copy=False)
    node_elems = inputs["node_elems"]
    weights, gtabs = _prep_weights(inputs)
    out, _ = run_sharded(node_feats, node_elems, weights, gtabs,
                         node_feats.shape[0])
    return out
